# revision 13
# baseline (speedup 1.0000x reference)
"""Trainium2 Bass kernel for nn_NCFG_21139829031662 (gnn_message_passing).

RippleNet-style model: hop-0 seed-set sum + 2 hops of (gather triples,
attention softmax over K, 2-step tanh RNN, weighted sum), then a
user/item dot + sigmoid.

Strategy: data-parallel over the 4096-user batch across 8 cores
(512 users/core); tables replicated per core. The critical resource is
the GpSimd (Pool) engine, which serializes SWDGE descriptor generation
for every indirect gather (~1.1us/call regardless of size). Changes vs
the f32 baseline:
  - entity table stored bf16 and augmented with the pre-multiplied RNN
    input term: row = [e (32), W_ih[:, :32] @ e (32)]. One 128B gather
    per head/tail feeds both the attention logits (raw half) and the
    RNN (pre-multiplied half, via PE transpose instead of a matmul).
  - relation gathers (2*B*K rows from a 64-row table) moved off the
    per-column indirect path onto the batched InstDMAGatherAnt
    primitive (int16 indices; one call per 16K tokens): row =
    [W_ih[:, 32:] @ r (32), |r|^2 (1), pad] at 256B.
  - item embedding table folded on host: entity[:200K] + rec_item.
  - all PE matmuls/transposes in bf16 (4x over f32).

Per-core on-chip layout ("G-layout"): token (u, k) -> partition
p = (u%2)*64 + k, free column j = u//2.
"""

import sys
import numpy as np

sys.path.insert(0, "/opt/trn_rl_repo")

# ---------------------------------------------------------------- constants
DIM = 32
N_ENTITY = 500000
N_RELATION = 64
N_USER = 100000
N_ITEM = 200000
B = 4096
K = 64
L = 2
NCORES = 8
P = 128


def build_core_program(BC=512, JB=32):
    """Build the single-core bass program (SPMD: same program on all cores).

    BC: users per core. JB: j-columns (user pairs) per compute batch.
    """
    import concourse.bass as bass
    import concourse.bacc as bacc
    import concourse.mybir as mybir
    import concourse.tile as tile
    from concourse.masks import make_identity

    J = BC // 2              # j-columns total
    NBATCH = J // JB         # compute batches per hop
    NCHUNK = J // 16         # 16-j output chunks
    NR = 2 * NCHUNK          # output psum rows
    assert J % JB == 0 and JB % 16 == 0
    CPB = JB // 16           # chunks per batch
    STB = JB // 4            # supertiles ([*,128] transpose blocks) per batch
    JH = J // 4              # j-columns per relation dma_gather call
    NIDX = JH * P            # idxs per relation call
    BPH = JH // JB           # compute batches per relation half
    f32 = mybir.dt.float32
    bf16 = mybir.dt.bfloat16
    i32 = mybir.dt.int32
    i16 = mybir.dt.int16

    nc = bacc.Bacc("TRN2", target_bir_lowering=False, debug=False,
                   num_swdge_queues=4)

    # DRAM inputs
    ent_aug = nc.dram_tensor("ent_aug", [N_ENTITY, 2 * DIM], bf16,
                             kind="ExternalInput").ap()
    iota64_in = nc.dram_tensor("iota64", [P, N_RELATION], f32,
                               kind="ExternalInput").ap()
    w4_in = nc.dram_tensor("w4", [P, 64], bf16, kind="ExternalInput").ap()
    rn2_in = nc.dram_tensor("rn2", [P, N_RELATION], f32,
                            kind="ExternalInput").ap()
    user_tab = nc.dram_tensor("user_tab", [N_USER, DIM], bf16,
                              kind="ExternalInput").ap()
    item_tab = nc.dram_tensor("item_tab", [N_ITEM, DIM], bf16,
                              kind="ExternalInput").ap()
    idx_hop0 = nc.dram_tensor("idx_hop0", [P, J], i32, kind="ExternalInput").ap()
    idx_h = nc.dram_tensor("idx_h", [L, P, J], i32, kind="ExternalInput").ap()
    idx_t = nc.dram_tensor("idx_t", [L, P, J], i32, kind="ExternalInput").ap()
    idx_rf = nc.dram_tensor("idx_rf", [L, P, J], f32, kind="ExternalInput").ap()
    fin_users = nc.dram_tensor("fin_users", [NR, 16], i32, kind="ExternalInput").ap()
    fin_items = nc.dram_tensor("fin_items", [NR, 16], i32, kind="ExternalInput").ap()
    whh_bd = nc.dram_tensor("whh_bd", [P, P], bf16, kind="ExternalInput").ap()
    b2_in = nc.dram_tensor("b2", [P, 1], f32, kind="ExternalInput").ap()
    sels_in = nc.dram_tensor("sels", [P, NCHUNK * NR], bf16,
                             kind="ExternalInput").ap()
    par2_in = nc.dram_tensor("par2", [P, 2], bf16, kind="ExternalInput").ap()
    parT_in = nc.dram_tensor("parT", [2, P], bf16, kind="ExternalInput").ap()
    out_dram = nc.dram_tensor("scores", [NR, 16], f32, kind="ExternalOutput").ap()

    with tile.TileContext(nc) as tc:
        with (
            tc.tile_pool(name="const", bufs=1) as cpool,
            tc.tile_pool(name="idx", bufs=4) as ipool,
            tc.tile_pool(name="gath", bufs=3) as gpool,
            tc.tile_pool(name="work", bufs=2) as wpool,
            tc.tile_pool(name="small", bufs=2) as spool,
            tc.tile_pool(name="psO", bufs=1, space="PSUM") as poolO,
            tc.tile_pool(name="psT", bufs=1, space="PSUM") as poolT,
            tc.tile_pool(name="psU", bufs=1, space="PSUM") as poolU,
            tc.tile_pool(name="psW", bufs=1, space="PSUM") as poolW,
            tc.tile_pool(name="psS", bufs=1, space="PSUM") as poolS,
        ):
            # ---------------- constants to SBUF
            ident = cpool.tile([P, P], bf16, tag="ident")
            make_identity(nc, ident[:])
            whh_t = cpool.tile([P, P], bf16, tag="whh")
            nc.sync.dma_start(out=whh_t[:], in_=whh_bd[:, :])
            b2_t = cpool.tile([P, 1], f32, tag="b2")
            nc.sync.dma_start(out=b2_t[:], in_=b2_in[:, :])
            sels_t = cpool.tile([P, NCHUNK * NR], bf16, tag="sels")
            nc.sync.dma_start(out=sels_t[:], in_=sels_in[:, :])
            par2_t = cpool.tile([P, 2], bf16, tag="par2")
            nc.sync.dma_start(out=par2_t[:], in_=par2_in[:, :])
            parT_t = cpool.tile([2, P], bf16, tag="parT")
            nc.sync.dma_start(out=parT_t[:], in_=parT_in[:, :])
            iota_t = cpool.tile([P, N_RELATION], f32, tag="iota")
            nc.sync.dma_start(out=iota_t[:], in_=iota64_in[:, :])
            w4_t = cpool.tile([P, 64], bf16, tag="w4")
            nc.sync.dma_start(out=w4_t[:], in_=w4_in[:, :])
            rn2_t = cpool.tile([P, N_RELATION], f32, tag="rn2")
            nc.sync.dma_start(out=rn2_t[:], in_=rn2_in[:, :])

            qn = [0]

            def igather(out_ap2, table_ap, off_ap):
                inst = nc.gpsimd.indirect_dma_start(
                    out=out_ap2, out_offset=None, in_=table_ap,
                    in_offset=bass.IndirectOffsetOnAxis(ap=off_ap, axis=0))
                q = qn[0] % 4
                if q:
                    inst.ins.queue = f"qPoolDynamic{q}"
                qn[0] += 1
                return inst

            # persistent output accumulator [NR, 512] (one PSUM bank)
            o_ps = poolO.tile([NR, 512], f32, tag="o")
            first_omm = [True]

            def o_accum(rhs_ap, chunk, is_last):
                """rhs [128, 512] bf16 -> accumulate selector chunk into o_ps."""
                nc.tensor.matmul(
                    out=o_ps[:, :],
                    lhsT=sels_t[:, chunk * NR:(chunk + 1) * NR],
                    rhs=rhs_ap,
                    start=first_omm[0],
                    stop=is_last,
                    skip_group_check=True,
                )
                first_omm[0] = False

            # ---------------- hop 0: gather raw e (64B prefix) + selector-sum
            for b in range(NBATCH):
                idx_t0 = ipool.tile([P, JB], i32, tag="i0")
                nc.sync.dma_start(out=idx_t0[:], in_=idx_hop0[:, b * JB:(b + 1) * JB])
                g0 = gpool.tile([P, JB * DIM], bf16, tag="g0")
                for jj in range(JB):
                    igather(g0[:, jj * DIM:(jj + 1) * DIM], ent_aug[:, :],
                            idx_t0[:, jj:jj + 1])
                for c in range(CPB):
                    o_accum(g0[:, c * 512:(c + 1) * 512], b * CPB + c, False)

            # ---------------- hops (relations via on-chip one-hot, no DMA)
            for l in range(L):
                for b in range(NBATCH):
                    jlo = b * JB
                    # index tiles + per-column gathers of [e | Wh@e] rows
                    ih = ipool.tile([P, JB], i32, tag="ih")
                    nc.sync.dma_start(out=ih[:], in_=idx_h[l, :, jlo:jlo + JB])
                    it = ipool.tile([P, JB], i32, tag="it")
                    nc.sync.dma_start(out=it[:], in_=idx_t[l, :, jlo:jlo + JB])
                    irf = ipool.tile([P, JB], f32, tag="irf")
                    nc.sync.dma_start(out=irf[:], in_=idx_rf[l, :, jlo:jlo + JB])
                    Hg = gpool.tile([P, JB * 2 * DIM], bf16, tag="h")
                    Tg = gpool.tile([P, JB * 2 * DIM], bf16, tag="t")
                    for jj in range(JB):
                        sl = slice(jj * 2 * DIM, (jj + 1) * 2 * DIM)
                        igather(Hg[:, sl], ent_aug[:, :], ih[:, jj:jj + 1])
                        igather(Tg[:, sl], ent_aug[:, :], it[:, jj:jj + 1])

                    hv = Hg[:, :].rearrange("p (j e) -> p j e", e=2 * DIM)
                    tv = Tg[:, :].rearrange("p (j e) -> p j e", e=2 * DIM)
                    whe_h = wpool.tile([P, JB * DIM], bf16, tag="wheh")
                    nc.scalar.copy(
                        out=whe_h[:].rearrange("p (j d) -> p j d", d=DIM),
                        in_=hv[:, :, DIM:2 * DIM])
                    whe_t = wpool.tile([P, JB * DIM], bf16, tag="whet")
                    nc.scalar.copy(
                        out=whe_t[:].rearrange("p (j d) -> p j d", d=DIM),
                        in_=tv[:, :, DIM:2 * DIM])

                    # ---- relation one-hot [p, (j, c)] and rnorm2
                    oh = wpool.tile([P, JB * N_RELATION], bf16, tag="oh")
                    ohv = oh[:, :].rearrange("p (j c) -> p j c", c=N_RELATION)
                    nc.vector.tensor_tensor(
                        out=ohv,
                        in0=irf[:, :, None].to_broadcast([P, JB, N_RELATION]),
                        in1=iota_t[:, None, :].to_broadcast([P, JB, N_RELATION]),
                        op=mybir.AluOpType.is_equal)
                    rtmp = wpool.tile([P, JB * N_RELATION], f32, tag="rtmp")
                    nc.vector.tensor_tensor(
                        out=rtmp[:].rearrange("p (j c) -> p j c", c=N_RELATION),
                        in0=ohv,
                        in1=rn2_t[:, None, :].to_broadcast([P, JB, N_RELATION]),
                        op=mybir.AluOpType.mult)
                    rn2b = spool.tile([P, JB], f32, tag="rn2b")
                    nc.vector.tensor_reduce(
                        out=rn2b[:],
                        in_=rtmp[:].rearrange("p (j c) -> p j c", c=N_RELATION),
                        axis=mybir.AxisListType.X, op=mybir.AluOpType.add)
                    # transpose one-hot to [(2j, c), tok] blocks
                    ohT_ps = poolU.tile([P, JB * N_RELATION], bf16, tag="ohT")
                    for blk in range(JB * N_RELATION // 128):
                        osl = slice(blk * 128, (blk + 1) * 128)
                        nc.tensor.transpose(
                            out=ohT_ps[:, osl], in_=oh[:, osl], identity=ident[:])
                    ohT = wpool.tile([P, JB * N_RELATION], bf16, tag="ohTs")
                    nc.scalar.copy(out=ohT[:], in_=ohT_ps[:])

                    # ---- logits: sum_d h*t + rnorm2[r]; pi = softmax_j-col
                    prod = wpool.tile([P, JB * DIM], f32, tag="prod")
                    nc.vector.tensor_tensor(
                        out=prod[:].rearrange("p (j d) -> p j d", d=DIM),
                        in0=hv[:, :, 0:DIM], in1=tv[:, :, 0:DIM],
                        op=mybir.AluOpType.mult)
                    dht = spool.tile([P, JB], f32, tag="dht")
                    nc.vector.tensor_reduce(
                        out=dht[:],
                        in_=prod[:].rearrange("p (j d) -> p j d", d=DIM),
                        axis=mybir.AxisListType.X, op=mybir.AluOpType.add)
                    logits = spool.tile([P, JB], f32, tag="lg")
                    nc.vector.tensor_tensor(
                        out=logits[:], in0=dht[:], in1=rn2b[:],
                        op=mybir.AluOpType.add)
                    E = spool.tile([P, JB], bf16, tag="E")
                    nc.scalar.activation(
                        out=E[:], in_=logits[:],
                        func=mybir.ActivationFunctionType.Exp)
                    den_ps = poolS.tile([2, JB], f32, tag="dn")
                    nc.tensor.matmul(out=den_ps[:], lhsT=par2_t[:], rhs=E[:],
                                     start=True, stop=True)
                    rec = spool.tile([2, JB], bf16, tag="rec")
                    with nc.allow_low_precision(reason="softmax denom bf16 ok"):
                        nc.vector.reciprocal(out=rec[:], in_=den_ps[:])
                    rb_ps = poolS.tile([P, JB], f32, tag="rb")
                    nc.tensor.matmul(out=rb_ps[:], lhsT=parT_t[:], rhs=rec[:],
                                     start=True, stop=True)
                    pi = spool.tile([P, JB], f32, tag="pi")
                    nc.vector.tensor_tensor(
                        out=pi[:], in0=E[:], in1=rb_ps[:],
                        op=mybir.AluOpType.mult)

                    # ---- RNN step 1: A = WhH_T + WrR_T (one PSUM group)
                    a_ps = poolW.tile([P, JB * DIM], f32, tag="ab")
                    for st in range(STB):
                        osl = slice(st * 128, (st + 1) * 128)
                        nc.tensor.matmul(
                            out=a_ps[:, osl],
                            lhsT=whe_h[:, osl],
                            rhs=ident[:], start=True, stop=False,
                            skip_group_check=True)
                        for hf in range(2):
                            nc.tensor.matmul(
                                out=a_ps[64 * hf:64 * (hf + 1), osl],
                                lhsT=w4_t[:],
                                rhs=ohT[:, (2 * st + hf) * 128:
                                        (2 * st + hf + 1) * 128],
                                start=False, stop=(hf == 1),
                                skip_group_check=True)
                    h1T = wpool.tile([P, JB * DIM], bf16, tag="h1T")
                    nc.scalar.activation(
                        out=h1T[:], in_=a_ps[:],
                        func=mybir.ActivationFunctionType.Tanh, bias=b2_t[:, :])

                    # ---- RNN step 2: B = WhT_T + WrR_T + Whh@h1T (one group)
                    b_ps = poolW.tile([P, JB * DIM], f32, tag="ab")
                    for st in range(STB):
                        osl = slice(st * 128, (st + 1) * 128)
                        nc.tensor.matmul(
                            out=b_ps[:, osl],
                            lhsT=whe_t[:, osl],
                            rhs=ident[:], start=True, stop=False,
                            skip_group_check=True)
                        for hf in range(2):
                            nc.tensor.matmul(
                                out=b_ps[64 * hf:64 * (hf + 1), osl],
                                lhsT=w4_t[:],
                                rhs=ohT[:, (2 * st + hf) * 128:
                                        (2 * st + hf + 1) * 128],
                                start=False, stop=False,
                                skip_group_check=True)
                        nc.tensor.matmul(
                            out=b_ps[:, osl], lhsT=whh_t[:], rhs=h1T[:, osl],
                            start=False, stop=True, skip_group_check=True)
                    h2T = wpool.tile([P, JB * DIM], bf16, tag="h2T")
                    nc.scalar.activation(
                        out=h2T[:], in_=b_ps[:],
                        func=mybir.ActivationFunctionType.Tanh, bias=b2_t[:, :])

                    # ---- back to token-major, scale by pi, accumulate into o
                    c_ps = poolT.tile([P, JB * DIM], bf16, tag="tpA")
                    for st in range(STB):
                        osl = slice(st * 128, (st + 1) * 128)
                        nc.tensor.transpose(
                            out=c_ps[:, osl], in_=h2T[:, osl],
                            identity=ident[:])
                    scaled = wpool.tile([P, JB * DIM], bf16, tag="sc")
                    nc.vector.tensor_tensor(
                        out=scaled[:].rearrange("p (j d) -> p j d", d=DIM),
                        in0=c_ps[:].rearrange("p (j d) -> p j d", d=DIM),
                        in1=pi[:, :, None].to_broadcast([P, JB, DIM]),
                        op=mybir.AluOpType.mult)
                    last = (l == L - 1) and (b == NBATCH - 1)
                    for c in range(CPB):
                        o_accum(scaled[:, c * 512:(c + 1) * 512], b * CPB + c,
                                last and c == CPB - 1)

            # ------------- final: sigmoid((o + ru[users]) . (e+ri)[items])
            fu = ipool.tile([NR, 16], i32, tag="fu")
            nc.sync.dma_start(out=fu[:], in_=fin_users[:, :])
            fi = ipool.tile([NR, 16], i32, tag="fi")
            nc.sync.dma_start(out=fi[:], in_=fin_items[:, :])
            ru_g = spool.tile([NR, 512], bf16, tag="ru")
            it_g = spool.tile([NR, 512], bf16, tag="ig")
            for jj in range(16):
                sl = slice(jj * DIM, (jj + 1) * DIM)
                igather(ru_g[:, sl], user_tab[:, :], fu[:, jj:jj + 1])
                igather(it_g[:, sl], item_tab[:, :], fi[:, jj:jj + 1])
            ue = spool.tile([NR, 512], f32, tag="ue")
            nc.vector.tensor_tensor(out=ue[:], in0=o_ps[:], in1=ru_g[:],
                                    op=mybir.AluOpType.add)
            pr = spool.tile([NR, 512], f32, tag="pr")
            nc.vector.tensor_tensor(out=pr[:], in0=ue[:], in1=it_g[:],
                                    op=mybir.AluOpType.mult)
            sc = spool.tile([NR, 16], f32, tag="scs")
            nc.vector.tensor_reduce(
                out=sc[:], in_=pr[:].rearrange("p (j d) -> p j d", d=DIM),
                axis=mybir.AxisListType.X, op=mybir.AluOpType.add)
            sg = spool.tile([NR, 16], f32, tag="sg")
            nc.scalar.activation(out=sg[:], in_=sc[:],
                                 func=mybir.ActivationFunctionType.Sigmoid)
            nc.sync.dma_start(out=out_dram[:, :], in_=sg[:])

    nc.compile()
    return nc


# ---------------------------------------------------------------- host prep
def _prep_tables(entity_emb, relation_emb, rec_user_emb, rec_item_emb,
                 W_ih, W_hh, b_ih, b_hh):
    """Shared (per-core-identical) table prep; pure weight folding."""
    import ml_dtypes
    bf = ml_dtypes.bfloat16
    Wh = W_ih[:, :DIM]           # [32, 32], h1_d = sum_e hr_e W_ih[d, e]
    Wr = W_ih[:, DIM:]
    ent_aug = np.concatenate(
        [entity_emb, entity_emb @ Wh.T], axis=1).astype(bf)
    premul = (relation_emb @ Wr.T).astype(np.float32)      # [64, 32]
    w4 = np.zeros((P, 64), np.float32)
    w4[0:64, 0:32] = premul
    w4[64:128, 32:64] = premul
    iota64 = np.tile(np.arange(N_RELATION, dtype=np.float32), (P, 1))
    rn2 = np.tile((relation_emb ** 2).sum(axis=1).astype(np.float32), (P, 1))
    user_tab = rec_user_emb.astype(bf)
    item_tab = (entity_emb[:N_ITEM] + rec_item_emb).astype(bf)

    def blockdiag(w):  # [32, 32] block = w.T
        m = np.zeros((P, P), np.float32)
        for j in range(4):
            m[j * 32:(j + 1) * 32, j * 32:(j + 1) * 32] = w.T
        return m

    b2 = np.tile((b_ih + b_hh).astype(np.float32), 4)[:, None]
    return {
        "ent_aug": ent_aug, "user_tab": user_tab, "item_tab": item_tab,
        "iota64": iota64, "w4": w4.astype(bf), "rn2": rn2,
        "whh_bd": blockdiag(W_hh).astype(bf),
        "b2": np.ascontiguousarray(b2),
    }


def _prep_core_inputs(c, BC, tables, users, items, hop0_items, heads,
                      relations, tails):
    """Per-core: shard + index-layout permutations + selector matrices."""
    import ml_dtypes
    bf = ml_dtypes.bfloat16
    J = BC // 2
    NCHUNK = J // 16
    NR = 2 * NCHUNK
    lo, hi = c * BC, (c + 1) * BC

    def glayout(a):  # [BC, K] -> [128, J]
        return np.ascontiguousarray(
            a.reshape(J, 2, K).transpose(1, 2, 0).reshape(P, J)).astype(np.int32)

    def flayout(a):  # [BC] -> [NR, 16]
        return np.ascontiguousarray(
            a.reshape(NCHUNK, 16, 2).transpose(0, 2, 1).reshape(NR, 16)).astype(np.int32)

    idx_h = np.stack([glayout(heads[l, lo:hi]) for l in range(L)])
    idx_t = np.stack([glayout(tails[l, lo:hi]) for l in range(L)])

    idx_rf = np.stack(
        [glayout(relations[l, lo:hi]).astype(np.float32) for l in range(L)])

    sels = np.zeros((P, NCHUNK, NR), np.float32)
    pvec = np.arange(P) // 64
    for m in range(NCHUNK):
        for p in range(P):
            sels[p, m, 2 * m + pvec[p]] = 1.0
    par2 = np.zeros((P, 2), np.float32)
    par2[np.arange(P), pvec] = 1.0

    out = dict(tables)
    out.update({
        "idx_hop0": glayout(hop0_items[lo:hi]),
        "idx_h": idx_h, "idx_t": idx_t, "idx_rf": idx_rf,
        "fin_users": flayout(users[lo:hi]),
        "fin_items": flayout(items[lo:hi]),
        "sels": np.ascontiguousarray(sels.reshape(P, NCHUNK * NR)).astype(bf),
        "par2": par2.astype(bf),
        "parT": np.ascontiguousarray(par2.T).astype(bf),
    })
    return out


def _unscramble(out_c, BC):
    """[NR, 16] core output -> [BC] user scores."""
    NCHUNK = (BC // 2) // 16
    return np.ascontiguousarray(
        out_c.reshape(NCHUNK, 2, 16).transpose(0, 2, 1).reshape(BC))


_CACHED = {}
TRACE = False
LAST_RESULTS = None


def kernel(**inputs):
    global LAST_RESULTS
    from concourse import bass_utils

    BC = B // NCORES
    if "nc" not in _CACHED:
        _CACHED["nc"] = build_core_program(BC=BC)
    nc = _CACHED["nc"]

    args = {k: np.asarray(v) for k, v in inputs.items()}
    tables = _prep_tables(
        np.asarray(args["entity_emb"], np.float32),
        np.asarray(args["relation_emb"], np.float32),
        np.asarray(args["rec_user_emb"], np.float32),
        np.asarray(args["rec_item_emb"], np.float32),
        np.asarray(args["W_ih"], np.float32),
        np.asarray(args["W_hh"], np.float32),
        np.asarray(args["b_ih"], np.float32),
        np.asarray(args["b_hh"], np.float32),
    )
    in_maps = [
        _prep_core_inputs(
            c, BC, tables,
            args["users"], args["items"], args["hop0_items"], args["heads"],
            args["relations"], args["tails"],
        )
        for c in range(NCORES)
    ]
    res = bass_utils.run_bass_kernel_spmd(
        nc, in_maps, core_ids=list(range(NCORES)), trace=TRACE)
    LAST_RESULTS = res
    out = np.concatenate(
        [_unscramble(res.results[c]["scores"], BC) for c in range(NCORES)])
    return out
